# revision 15
# baseline (speedup 1.0000x reference)
"""AtomAttentionEncoder distributed Bass kernel for 8 TRN2 NeuronCores.

Atoms sharded 256/core. Channel-major (c-major) residual stream; pair MLP with
fused one-hot trunk add; block-sparse attention with all-gathered k/v and
dma_gather neighbor fetch; scatter-mean via one-hot matmul + AllReduce.
Outputs p_lm in [atom, c, j] layout (host transposes), c_atom0/q_atom
transposed on host.
"""
import numpy as np
import ml_dtypes

import concourse.bass as bass
import concourse.bacc as bacc
import concourse.mybir as mybir
import concourse.tile as tile
from concourse.bass_utils import run_bass_kernel_spmd

N_CORES = 8
N_ATOM, N_TOKEN, BLOCK = 2048, 256, 24
H, L, DH = 4, 3, 32
NA = N_ATOM // N_CORES          # 256 atoms/core
NBF = 128 * BLOCK               # 3072 neighbor pairs per atom-tile

FP32 = mybir.dt.float32
BF16 = mybir.dt.bfloat16
I16 = mybir.dt.int16
AF = mybir.ActivationFunctionType
ALU = mybir.AluOpType
AX = mybir.AxisListType

bf = ml_dtypes.bfloat16


def build():
    nc = bacc.Bacc("TRN2", target_bir_lowering=False, debug=False,
                   num_devices=N_CORES)
    P = {}
    specs = [
        ("xinT", (2, 128, NA), BF16), ("aW", (2, 128, 128), BF16),
        ("tsW", (2, 128, 128), BF16), ("oh_uloc", (2, 128, NA), BF16),
        ("bemb", (128, 1), FP32),
        ("posneg", (128, N_ATOM), BF16), ("pairsc", (128, NA // 32), FP32),
        ("wa_bd", (16, 128, 128), BF16), ("bacol", (128, 1), FP32),
        ("wm2", (2, 128, 128), BF16),
        ("trunkW", (2, 128, NA * 32), BF16), ("oh_u", (2, 128, N_ATOM), BF16),
        ("b2col", (128, 1), FP32), ("ident", (128, 128), BF16),
        ("identf", (128, 128), FP32),
        ("kvidx", (2, 128, NBF // 16), I16),
        ("pinnb", (4, 2 * NBF), BF16), ("trunknbT", (32, 2 * NBF), BF16),
        ("wanb", (4, 64), BF16), ("ba64", (64, 1), FP32),
        ("wm2nb", (64, 32), BF16),
        ("wbL", (32, 12), BF16), ("bbcol", (12, 1), FP32),
        ("lncols", (L, 4, 128, 1), FP32),
        ("wq", (L, 128, 128), BF16), ("wk", (L, 128, 128), BF16),
        ("wv", (L, 128, 128), BF16), ("wo", (L, 128, 128), BF16),
        ("w1", (L, 4, 128, 128), BF16), ("w2", (L, 4, 128, 128), BF16),
        ("bqkvo", (L, 4, 128, 1), FP32), ("b1col", (L, 4, 128, 1), FP32),
        ("b2acol", (L, 128, 1), FP32),
        ("postW", (3, 128, 128), BF16), ("postb", (3, 128, 1), FP32),
        ("oh_atok", (2, 128, N_TOKEN), BF16),
    ]
    for name, shape, dt in specs:
        P[name] = nc.declare_dram_parameter(name, list(shape), dt, isOutput=False)

    plm = nc.declare_dram_parameter("plm", [NA, 32, N_ATOM], FP32, isOutput=True)
    catom0T = nc.declare_dram_parameter("catom0T", [128, NA], FP32, isOutput=True)
    qatomT = nc.declare_dram_parameter("qatomT", [128, NA], FP32, isOutput=True)
    atok = nc.declare_dram_parameter("atok", [N_TOKEN, 384], FP32, isOutput=True)

    with tile.TileContext(nc, num_cores=N_CORES) as tc:
        with (
            tc.tile_pool(name="cst", bufs=1) as cst,
            tc.tile_pool(name="sb", bufs=2) as sb,
            tc.tile_pool(name="ps", bufs=2, space="PSUM") as ps,
            tc.tile_pool(name="dram", bufs=1, space="DRAM") as dram,
        ):
            C = {}
            TRANSIENT = {"pinnb", "trunknbT"}
            SKIP = {"trunkW"}
            for name, shape, dt in specs:
                if name in SKIP:
                    continue
                if len(shape) == 2:
                    pool_ = sb if name in TRANSIENT else cst
                    if name in TRANSIENT:
                        t = pool_.tile(list(shape), dt, name=f"c_{name}",
                                       tag="nbw", bufs=3)
                    else:
                        t = pool_.tile(list(shape), dt, name=f"c_{name}",
                                       bufs=1)
                    nc.sync.dma_start(t[:], P[name][:])
                    C[name] = t
                elif len(shape) == 3:
                    C[name] = []
                    for i in range(shape[0]):
                        t = cst.tile(list(shape[1:]), dt, name=f"c_{name}{i}")
                        nc.sync.dma_start(t[:], P[name][i])
                        C[name].append(t)
                else:
                    C[name] = []
                    for i in range(shape[0]):
                        row = []
                        for j in range(shape[1]):
                            t = cst.tile(list(shape[2:]), dt,
                                         name=f"c_{name}{i}_{j}")
                            nc.sync.dma_start(t[:], P[name][i, j])
                            row.append(t)
                        C[name].append(row)

            # ============ embed: c_atom0 ============
            cat_ps = ps.tile([128, NA], FP32, name="cat_ps", tag="gen")
            nc.tensor.matmul(cat_ps[:], C["aW"][0][:], C["xinT"][0][:], start=True,
                             stop=False)
            nc.tensor.matmul(cat_ps[:], C["aW"][1][:], C["xinT"][1][:], start=False,
                             stop=False)
            nc.tensor.matmul(cat_ps[:], C["tsW"][0][:], C["oh_uloc"][0][:],
                             start=False, stop=False)
            nc.tensor.matmul(cat_ps[:], C["tsW"][1][:], C["oh_uloc"][1][:],
                             start=False, stop=True)
            x_t = cst.tile([128, NA], FP32, name="x_t")
            nc.scalar.activation(x_t[:], cat_ps[:], AF.Identity,
                                 bias=C["bemb"][:])
            nc.sync.dma_start(catom0T[:], x_t[:])

            # ============ p_nb MLP + bias (c-major, then transpose) ============
            h1nb = sb.tile([64, 2 * NBF], BF16, name="h1nb", tag="nbw", bufs=3)
            for ch in range(12):
                sl = slice(ch * 512, (ch + 1) * 512)
                hps = ps.tile([64, 512], FP32, name=f"hnb{ch}", tag="gen")
                nc.tensor.matmul(hps[:], C["wanb"][:], C["pinnb"][:, sl],
                                 start=True, stop=True)
                nc.scalar.activation(h1nb[:, sl], hps[:], AF.Relu,
                                     bias=C["ba64"][:])
            pnb_t = sb.tile([32, 2 * NBF], BF16, name="pnb_t", tag="nbw", bufs=3)
            for ch in range(12):
                sl = slice(ch * 512, (ch + 1) * 512)
                pps = ps.tile([32, 512], FP32, name=f"pnb{ch}", tag="gen")
                nc.tensor.matmul(pps[:], C["wm2nb"][:], h1nb[:, sl],
                                 start=True, stop=True)
                nc.vector.tensor_tensor(pnb_t[:, sl], pps[:],
                                        C["trunknbT"][:, sl], op=ALU.add)
            biasL = sb.tile([12, 2 * NBF], BF16, name="biasL", tag="nbw", bufs=3)
            for ch in range(12):
                sl = slice(ch * 512, (ch + 1) * 512)
                bps = ps.tile([12, 512], FP32, name=f"bL{ch}", tag="gen")
                nc.tensor.matmul(bps[:], C["wbL"][:], pnb_t[:, sl],
                                 start=True, stop=True)
                nc.scalar.activation(biasL[:, sl], bps[:], AF.Identity,
                                     bias=C["bbcol"][:])
            # transpose [12, 128]-chunks -> bias_am [tile][128, 24, 12]
            bias_am = []
            for t in range(2):
                bam = cst.tile([128, BLOCK, 12], FP32, name=f"bam{t}")
                bias_am.append(bam)
                for b in range(BLOCK):
                    tb = ps.tile([128, 12], BF16, name=f"tb{t}_{b}", tag="gen")
                    nc.tensor.transpose(
                        tb[:], biasL[:, (t * BLOCK + b) * 128:
                                     (t * BLOCK + b) * 128 + 128],
                        C["ident"][0:12, 0:12])
                    nc.vector.tensor_copy(bam[:, b, :], tb[:])

            # ============ helpers ============
            def ln(x_in, scol, bcol, out16, pref):
                sq = sb.tile([128, NA], FP32, name=f"{pref}_sq", tag="lnsq")
                nc.vector.tensor_tensor(sq[:], x_in[:], x_in[:], op=ALU.mult)
                st = ps.tile([1, 2 * NA], FP32, name=f"{pref}_st", tag="gen")
                nc.tensor.matmul(st[:, 0:NA], onescol[:], x_in[:], start=True,
                                 stop=True)
                nc.tensor.matmul(st[:, NA:], onescol[:], sq[:], start=True,
                                 stop=True)
                stats = sb.tile([1, 2 * NA], FP32, name=f"{pref}_sts",
                                tag="lnsts")
                nc.vector.tensor_scalar(stats[:], st[:], 1.0 / 128, None,
                                        op0=ALU.mult)
                m2 = sb.tile([1, NA], FP32, name=f"{pref}_m2", tag="lnm2")
                nc.vector.tensor_tensor(m2[:], stats[:, 0:NA], stats[:, 0:NA],
                                        op=ALU.mult)
                var = sb.tile([1, NA], FP32, name=f"{pref}_var", tag="lnvar")
                nc.vector.tensor_tensor(var[:], stats[:, NA:], m2[:],
                                        op=ALU.subtract)
                sd = sb.tile([1, NA], FP32, name=f"{pref}_sd", tag="lnsd")
                nc.scalar.activation(sd[:], var[:], AF.Sqrt, bias=epscol[:])
                rstd = sb.tile([1, NA], FP32, name=f"{pref}_rs", tag="lnrs")
                nc.vector.reciprocal(rstd[:], sd[:])
                nm = sb.tile([1, NA], FP32, name=f"{pref}_nm", tag="lnnm")
                nc.vector.tensor_scalar(nm[:], stats[:, 0:NA], -1.0, None,
                                        op0=ALU.mult)
                rep = ps.tile([128, 2 * NA], FP32, name=f"{pref}_rep",
                              tag="gen")
                nc.tensor.matmul(rep[:, 0:NA], onesrow[:], nm[:], start=True,
                                 stop=True)
                nc.tensor.matmul(rep[:, NA:], onesrow[:], rstd[:], start=True,
                                 stop=True)
                xc = sb.tile([128, NA], FP32, name=f"{pref}_xc", tag="lnxc")
                nc.vector.tensor_tensor(xc[:], rep[:, 0:NA], x_in[:],
                                        op=ALU.add)
                xn = sb.tile([128, NA], FP32, name=f"{pref}_xn", tag="lnxn")
                nc.vector.tensor_tensor(xn[:], xc[:], rep[:, NA:], op=ALU.mult)
                nc.scalar.activation(out16[:], xn[:], AF.Identity,
                                     bias=bcol, scale=scol)

            epscol = cst.tile([1, 1], FP32, name="epscol")
            nc.gpsimd.memset(epscol[:], 1e-5)
            onescol = cst.tile([128, 1], FP32, name="onescol")
            nc.gpsimd.memset(onescol[:], 1.0)
            onesrow = cst.tile([1, 128], FP32, name="onesrow")
            nc.gpsimd.memset(onesrow[:], 1.0)

            def transpose_cm_to_am(src16, pref, n=2):
                """[128, NA] bf16 c-major -> list of [128,128] bf16 atom-major."""
                outs = []
                for t in range(n):
                    tp = ps.tile([128, 128], BF16, name=f"{pref}_tp{t}",
                                 tag="gen")
                    nc.tensor.transpose(tp[:], src16[:, t * 128:(t + 1) * 128],
                                        C["ident"][:])
                    o = sb.tile([128, 128], BF16, name=f"{pref}_am{t}",
                                tag=f"{pref}am")
                    nc.vector.tensor_copy(o[:], tp[:])
                    outs.append(o)
                return outs

            # kv DRAM buffers per layer
            kvb = [dram.tile([NA, 256], BF16, name=f"kvb{l}") for l in range(L)]
            kvf = [dram.tile([N_ATOM, 256], BF16, name=f"kvf{l}")
                   for l in range(L)]


            # ============ 3 attention layers ============
            for l in range(L):
                h16 = sb.tile([128, NA], BF16, name=f"h16_{l}", tag="h16")
                ln(x_t, C["lncols"][l][0][:], C["lncols"][l][1][:], h16, f"ln1_{l}")
                # qkv
                qt = sb.tile([128, NA], BF16, name=f"qt{l}", tag="qt")
                kt = sb.tile([128, NA], BF16, name=f"kt{l}", tag="kt")
                vt = sb.tile([128, NA], BF16, name=f"vt{l}", tag="vt")
                for (w, bi, dst) in ((C["wq"][l], 0, qt), (C["wk"][l], 1, kt),
                                     (C["wv"][l], 2, vt)):
                    pp = ps.tile([128, NA], FP32, name=f"qkv{l}_{bi}",
                                 tag="gen")
                    nc.tensor.matmul(pp[:], w[:], h16[:], start=True, stop=True)
                    nc.scalar.activation(dst[:], pp[:], AF.Identity,
                                         bias=C["bqkvo"][l][bi][:])
                q_am = transpose_cm_to_am(qt, f"q{l}")
                k_am = transpose_cm_to_am(kt, f"k{l}")
                v_am = transpose_cm_to_am(vt, f"v{l}")
                # assemble kv atom-major & bounce out
                for t in range(2):
                    kvt_ = sb.tile([128, 256], BF16, name=f"kvt{l}_{t}",
                                   tag="kvtas")
                    nc.vector.tensor_copy(kvt_[:, 0:128], k_am[t][:])
                    nc.vector.tensor_copy(kvt_[:, 128:256], v_am[t][:])
                    nc.sync.dma_start(kvb[l][t * 128:(t + 1) * 128, :], kvt_[:])
                nc.gpsimd.collective_compute(
                    "AllGather", ALU.bypass,
                    replica_groups=[list(range(N_CORES))],
                    ins=[kvb[l].opt()], outs=[kvf[l].opt()])
                # attention per atom-tile
                o_am = []
                for t in range(2):
                    gkv = sb.tile([128, BLOCK, 256], BF16,
                                  name=f"gkv{l}_{t}", tag=f"g{t}", bufs=2)
                    nc.gpsimd.dma_gather(gkv[:], kvf[l][:], C["kvidx"][t][:],
                                         num_idxs=NBF, num_idxs_reg=NBF,
                                         elem_size=256, single_packet=False)
                    prod = sb.tile([128, H, BLOCK, DH], BF16,
                                   name=f"pr{l}_{t}", tag="prod", bufs=1)
                    nc.vector.tensor_tensor(
                        prod[:].rearrange("p h b d -> p b h d"),
                        gkv[:, :, 0:128].rearrange("p b (h d) -> p b h d", h=H),
                        q_am[t][:].rearrange("p (o h d) -> p o h d", o=1,
                                             h=H).to_broadcast(
                            [128, BLOCK, H, DH]),
                        op=ALU.mult)
                    s1 = sb.tile([128, H * BLOCK], FP32, name=f"s1{l}_{t}",
                                 tag="s1")
                    nc.vector.tensor_reduce(
                        s1[:], prod[:].rearrange("p h b d -> p (h b) d"),
                        axis=AX.X, op=ALU.add)
                    sc = sb.tile([128, H, BLOCK], FP32, name=f"sc{l}_{t}",
                                 tag="sc")
                    nc.vector.tensor_tensor(
                        sc[:],
                        s1[:].rearrange("p (h b) -> p h b", h=H),
                        bias_am[t][:].rearrange("p b lh -> p lh b")[
                            :, l * 4:(l + 1) * 4, :],
                        op=ALU.add)
                    ex = sb.tile([128, H, BLOCK], FP32, name=f"ex{l}_{t}",
                                 tag="ex")
                    nc.scalar.activation(ex[:], sc[:], AF.Exp)
                    sm = sb.tile([128, H], FP32, name=f"sm{l}_{t}", tag="sm")
                    nc.vector.tensor_reduce(sm[:], ex[:], axis=AX.X, op=ALU.add)
                    rec = sb.tile([128, H], FP32, name=f"rc{l}_{t}", tag="rc")
                    nc.vector.reciprocal(rec[:], sm[:])
                    at = sb.tile([128, H, BLOCK], FP32, name=f"at{l}_{t}",
                                 tag="at")
                    nc.vector.tensor_tensor(
                        at[:], ex[:],
                        rec[:].rearrange("p (h o) -> p h o", o=1).to_broadcast(
                            [128, H, BLOCK]),
                        op=ALU.mult)
                    pr2 = sb.tile([128, H, DH, BLOCK], BF16, name=f"p2{l}_{t}",
                                  tag="prod", bufs=1)
                    nc.vector.tensor_tensor(
                        pr2[:].rearrange("p h d b -> p b h d"),
                        gkv[:, :, 128:256].rearrange("p b (h d) -> p b h d",
                                                     h=H),
                        at[:].rearrange("p h (b o) -> p b h o",
                                        o=1).to_broadcast(
                            [128, BLOCK, H, DH]),
                        op=ALU.mult)
                    oam = sb.tile([128, H * DH], FP32, name=f"oa{l}_{t}",
                                  tag="oam")
                    nc.vector.tensor_reduce(
                        oam[:], pr2[:].rearrange("p h d b -> p (h d) b"),
                        axis=AX.X, op=ALU.add)
                    o_am.append(oam)
                # o -> c-major, Wo, residual
                ocm = sb.tile([128, NA], BF16, name=f"ocm{l}", tag="ocm")
                for t in range(2):
                    op_ = ps.tile([128, 128], FP32, name=f"otp{l}_{t}",
                                  tag="gen")
                    nc.tensor.transpose(op_[:], o_am[t][:], C["identf"][:])
                    nc.vector.tensor_copy(ocm[:, t * 128:(t + 1) * 128], op_[:])
                rps = ps.tile([128, NA], FP32, name=f"rps{l}", tag="gen")
                nc.tensor.matmul(rps[:], C["wo"][l][:], ocm[:], start=True,
                                 stop=True)
                xb = sb.tile([128, NA], FP32, name=f"xb{l}", tag="xb")
                nc.scalar.activation(xb[:], rps[:], AF.Identity,
                                     bias=C["bqkvo"][l][3][:])
                nc.vector.tensor_tensor(x_t[:], x_t[:], xb[:], op=ALU.add)
                # mlp
                h2 = sb.tile([128, NA], BF16, name=f"h2_{l}", tag="h16")
                ln(x_t, C["lncols"][l][2][:], C["lncols"][l][3][:], h2, f"ln2_{l}")
                mh = sb.tile([128, 4 * NA], BF16, name=f"mh{l}", tag="mh")
                for m in range(4):
                    mp = ps.tile([128, NA], FP32, name=f"mp{l}_{m}", tag="gen")
                    nc.tensor.matmul(mp[:], C["w1"][l][m][:], h2[:], start=True,
                                     stop=True)
                    nc.scalar.activation(mh[:, m * NA:(m + 1) * NA], mp[:],
                                         AF.Relu, bias=C["b1col"][l][m][:])
                m2ps = ps.tile([128, NA], FP32, name=f"m2ps{l}", tag="gen")
                for k in range(4):
                    nc.tensor.matmul(m2ps[:], C["w2"][l][k][:],
                                     mh[:, k * NA:(k + 1) * NA],
                                     start=(k == 0), stop=(k == 3))
                xb2 = sb.tile([128, NA], FP32, name=f"xb2{l}", tag="xb")
                nc.scalar.activation(xb2[:], m2ps[:], AF.Identity,
                                     bias=C["b2acol"][l][:])
                nc.vector.tensor_tensor(x_t[:], x_t[:], xb2[:], op=ALU.add)

            # ============ outputs: q_atom, a_token ============
            nc.sync.dma_start(qatomT[:], x_t[:])
            x16f = sb.tile([128, NA], BF16, name="x16f")
            nc.vector.tensor_copy(x16f[:], x_t[:])
            qp_am = []
            for t in range(2):
                qpt = sb.tile([128, 385], BF16, name=f"qpam{t}")
                nc.vector.tensor_copy(qpt[:, 384:385],
                                      onescol[:].bitcast(FP32) if False
                                      else onescol[:])
                qp_am.append(qpt)
            for m in range(3):
                pp = ps.tile([128, NA], FP32, name=f"qproj{m}", tag="gen")
                nc.tensor.matmul(pp[:], C["postW"][m][:], x16f[:], start=True,
                                 stop=True)
                qps = sb.tile([128, NA], BF16, name=f"qps{m}", tag="qps")
                nc.scalar.activation(qps[:], pp[:], AF.Identity,
                                     bias=C["postb"][m][:])
                for t in range(2):
                    tp = ps.tile([128, 128], BF16, name=f"qtp{m}_{t}",
                                 tag="gen")
                    nc.tensor.transpose(tp[:], qps[:, t * 128:(t + 1) * 128],
                                        C["ident"][:])
                    nc.vector.tensor_copy(
                        qp_am[t][:, m * 128:(m + 1) * 128], tp[:])
            arb = dram.tile([2, 128, 385], FP32, name="arb")
            aro = dram.tile([2, 128, 385], FP32, name="aro")
            for mt in range(2):
                sp = ps.tile([128, 385], FP32, name=f"sump{mt}", tag="gen")
                for kc in range(2):
                    nc.tensor.matmul(
                        sp[:], C["oh_atok"][kc][:, mt * 128:(mt + 1) * 128],
                        qp_am[kc][:], start=(kc == 0), stop=(kc == 1))
                ss = sb.tile([128, 385], FP32, name=f"sums{mt}", tag="sums")
                nc.vector.tensor_copy(ss[:], sp[:])
                nc.sync.dma_start(arb[mt], ss[:])
            nc.gpsimd.collective_compute(
                "AllReduce", ALU.add, replica_groups=[list(range(N_CORES))],
                ins=[arb.opt()], outs=[aro.opt()])
            for mt in range(2):
                sr = sb.tile([128, 385], FP32, name=f"sred{mt}", tag="sums")
                nc.sync.dma_start(sr[:], aro[mt])
                cnt = sb.tile([128, 1], FP32, name=f"cnt{mt}", tag="cnt")
                nc.vector.tensor_scalar(cnt[:], sr[:, 384:385], 1.0, None,
                                        op0=ALU.max)
                rcc = sb.tile([128, 1], FP32, name=f"rcc{mt}", tag="rcc")
                nc.vector.reciprocal(rcc[:], cnt[:])
                av = sb.tile([128, 384], FP32, name=f"av{mt}", tag="av")
                nc.vector.tensor_scalar(av[:], sr[:, 0:384], rcc[:], None,
                                        op0=ALU.mult)
                nc.sync.dma_start(atok[mt * 128:(mt + 1) * 128, :], av[:])

            # ============ pair pipeline (the bulk) ============
            for grp in range(NA // 32):
                pi = sb.tile([128, N_ATOM], BF16, name=f"pi{grp}", tag="pi")
                nc.vector.tensor_scalar(pi[0:96, :], C["posneg"][0:96, :],
                                        C["pairsc"][0:96, grp:grp + 1], None,
                                        op0=ALU.add)
                nc.vector.tensor_scalar(pi[96:128, :], C["posneg"][96:128, :],
                                        C["pairsc"][96:128, grp:grp + 1], None,
                                        op0=ALU.is_equal)
                for blk in range(8):
                    a0 = grp * 32 + blk * 4  # first atom (local) of block
                    csl0 = slice(a0 * 32, a0 * 32 + 128)
                    tw = []
                    for kc in range(2):
                        twt = sb.tile([128, 128], BF16,
                                      name=f"tw{grp}_{blk}_{kc}",
                                      tag=f"tw{kc}")
                        nc.sync.dma_start(twt[:], P["trunkW"][kc][:, csl0])
                        tw.append(twt)
                    for jc in range(4):
                        jsl = slice(jc * 512, (jc + 1) * 512)
                        h1a = ps.tile([128, 512], FP32,
                                      name=f"h1a{grp}_{blk}_{jc}", tag="h1a",
                                      bufs=2)
                        h1b = ps.tile([128, 512], FP32,
                                      name=f"h1b{grp}_{blk}_{jc}", tag="h1b",
                                      bufs=2)
                        nc.tensor.matmul(h1a[:], C["wa_bd"][blk * 2][:],
                                         pi[:, jsl], start=True, stop=True)
                        nc.tensor.matmul(h1b[:], C["wa_bd"][blk * 2 + 1][:],
                                         pi[:, jsl], start=True, stop=True)
                        h1as = sb.tile([128, 512], BF16,
                                       name=f"h1as{grp}_{blk}_{jc}", tag="h1as")
                        nc.scalar.activation(h1as[:], h1a[:], AF.Relu,
                                             bias=C["bacol"][:])
                        h1bs = sb.tile([128, 512], BF16,
                                       name=f"h1bs{grp}_{blk}_{jc}", tag="h1bs")
                        nc.scalar.activation(h1bs[:], h1b[:], AF.Relu,
                                             bias=C["bacol"][:])
                        p2 = ps.tile([128, 512], FP32,
                                     name=f"p2{grp}_{blk}_{jc}", tag="p2",
                                     bufs=2)
                        nc.tensor.matmul(p2[:], C["wm2"][0][:], h1as[:],
                                         start=True, stop=False)
                        nc.tensor.matmul(p2[:], C["wm2"][1][:], h1bs[:],
                                         start=False, stop=False)
                        nc.tensor.matmul(p2[:], tw[0][:], C["oh_u"][0][:, jsl],
                                         start=False, stop=False)
                        nc.tensor.matmul(p2[:], tw[1][:], C["oh_u"][1][:, jsl],
                                         start=False, stop=True)
                        po = sb.tile([128, 512], FP32,
                                     name=f"po{grp}_{blk}_{jc}", tag="po")
                        nc.vector.tensor_scalar(po[:], p2[:], C["b2col"][:],
                                                None, op0=ALU.add)
                        nc.sync.dma_start(
                            plm[:].rearrange("a c j -> (a c) j")[
                                a0 * 32:a0 * 32 + 128, jsl], po[:])

    nc.compile()
    return nc


_CACHE = {}


def kernel(ref_pos, ref_charge, ref_element, ref_atom_name_chars, atom_to_token,
           restype, trunk_sing, trunk_pair, block_index, params):
    Pm = {k: np.asarray(v, np.float32) for k, v in params.items()}
    pos = np.asarray(ref_pos, np.float32)
    a2t = np.asarray(atom_to_token, np.int64).astype(np.int32)
    bidx = np.asarray(block_index, np.int64).astype(np.int32)
    tsing = np.asarray(trunk_sing, np.float32)
    tpair = np.asarray(trunk_pair, np.float32)
    inv = np.float32(1.0 / np.sqrt(DH))

    x_in = np.concatenate(
        [pos, np.asarray(ref_charge, np.float32)[:, None],
         np.asarray(ref_element, np.float32),
         np.asarray(ref_atom_name_chars, np.float32)], axis=1)  # [2048, 148]

    if "nc" not in _CACHE:
        _CACHE["nc"] = build()
    nc = _CACHE["nc"]

    # fused stage-a weights: Wa = pair_W @ mlp1_W  [4, 64], ba = pair_b@mlp1+b1
    Wa = (Pm["pair_W"] @ Pm["mlp1_W"]).astype(np.float32)
    ba = (Pm["pair_b"] @ Pm["mlp1_W"] + Pm["mlp1_b"]).astype(np.float32)
    W2p = Pm["mlp2_W"]  # [64, 32]
    b2p = Pm["mlp2_b"]

    wa_bd = np.zeros((16, 128, 128), np.float32)
    for p in range(16):
        i0, i1 = 2 * p, 2 * p + 1
        for c in range(4):
            wa_bd[p, c * 32 + i0, 0:64] = Wa[c]
            wa_bd[p, c * 32 + i1, 64:128] = Wa[c]
    bacol = np.tile(ba, 2)[:, None]
    wm2 = np.zeros((2, 128, 128), np.float32)
    wm2[0, 0:64, 0:32] = W2p
    wm2[0, 64:128, 32:64] = W2p
    wm2[1, 0:64, 64:96] = W2p
    wm2[1, 64:128, 96:128] = W2p
    b2col = np.tile(b2p, 4)[:, None]

    onehot_u = np.zeros((256, N_ATOM), np.float32)
    onehot_u[a2t, np.arange(N_ATOM)] = 1.0

    identity = np.eye(128, dtype=np.float32)

    in_maps = []
    hosts = []
    for r in range(N_CORES):
        A = slice(r * NA, (r + 1) * NA)
        al = np.arange(r * NA, (r + 1) * NA)
        x_loc = x_in[A]  # [256, 148]
        xinT = np.zeros((2, 128, NA), np.float32)
        aW = np.zeros((2, 128, 128), np.float32)
        xt = x_loc.T  # [148, 256]
        xinT[0, :, :] = xt[0:128]
        xinT[1, 0:20, :] = xt[128:148]
        aW[0] = Pm["atom_W"][0:128]
        aW[1, 0:20] = Pm["atom_W"][128:148]
        tsW = np.stack([tsing[0:128], tsing[128:256]])  # [2,128,128]
        oh_uloc = np.zeros((2, 128, NA), np.float32)
        tloc = a2t[A]
        for a in range(NA):
            u = tloc[a]
            oh_uloc[u // 128, u % 128, a] = 1.0
        posneg = np.zeros((128, N_ATOM), np.float32)
        for c in range(3):
            posneg[c * 32:(c + 1) * 32, :] = -pos[:, c][None, :]
        posneg[96:128, :] = a2t[None, :].astype(np.float32)
        pairsc = np.zeros((128, NA // 32), np.float32)
        for g in range(NA // 32):
            ga = al[g * 32:(g + 1) * 32]
            for c in range(3):
                pairsc[c * 32:(c + 1) * 32, g] = pos[ga, c]
            pairsc[96:128, g] = a2t[ga]
        trunkW = np.zeros((2, 128, NA * 32), np.float32)
        for a in range(NA):
            t_a = a2t[al[a]]
            blk_ = tpair[t_a]  # [256, 32]
            trunkW[0, :, a * 32:(a + 1) * 32] = blk_[0:128]
            trunkW[1, :, a * 32:(a + 1) * 32] = blk_[128:256]
        # neighbor indices, flat order m = t*3072 + b*128 + p
        kvidx = np.zeros((2, 128, NBF // 16), np.int16)
        pinnb = np.zeros((4, 2 * NBF), np.float32)
        trunknbT = np.zeros((32, 2 * NBF), np.float32)
        for t in range(2):
            atoms = al[t * 128:(t + 1) * 128]
            flat = np.empty(NBF, np.int64)
            for b in range(BLOCK):
                flat[b * 128:(b + 1) * 128] = bidx[atoms, b]
            kvidx[t] = wrap_idx(flat)
            cols = slice(t * NBF, (t + 1) * NBF)
            arep = np.tile(atoms, BLOCK)
            barr = np.repeat(np.arange(BLOCK), 128)
            nb = bidx[arep, barr]
            pinnb[0:3, cols] = (pos[arep] - pos[nb]).T
            pinnb[3, cols] = (a2t[arep] == a2t[nb]).astype(np.float32)
            trunknbT[:, cols] = tpair[a2t[arep], a2t[nb]].T + b2p[:, None]
        wbL = np.zeros((32, 12), np.float32)
        bbcol = np.zeros((12, 1), np.float32)
        for l in range(L):
            wbL[:, l * 4:(l + 1) * 4] = Pm["Wb"][l]
            bbcol[l * 4:(l + 1) * 4, 0] = Pm["bb"][l]
        lncols = np.zeros((L, 4, 128, 1), np.float32)
        bqkvo = np.zeros((L, 4, 128, 1), np.float32)
        b1col = np.zeros((L, 4, 128, 1), np.float32)
        b2acol = np.zeros((L, 128, 1), np.float32)
        wqa = np.zeros((L, 128, 128), np.float32)
        wka = np.zeros((L, 128, 128), np.float32)
        wva = np.zeros((L, 128, 128), np.float32)
        woa = np.zeros((L, 128, 128), np.float32)
        w1a = np.zeros((L, 4, 128, 128), np.float32)
        w2a = np.zeros((L, 4, 128, 128), np.float32)
        for l in range(L):
            lncols[l, 0, :, 0] = Pm["ln1_s"][l]
            lncols[l, 1, :, 0] = Pm["ln1_b"][l]
            lncols[l, 2, :, 0] = Pm["ln2_s"][l]
            lncols[l, 3, :, 0] = Pm["ln2_b"][l]
            wqa[l] = Pm["Wq"][l] * inv
            wka[l] = Pm["Wk"][l]
            wva[l] = Pm["Wv"][l]
            woa[l] = Pm["Wo"][l]
            bqkvo[l, 0, :, 0] = Pm["bq"][l] * inv
            bqkvo[l, 1, :, 0] = Pm["bk"][l]
            bqkvo[l, 2, :, 0] = Pm["bv"][l]
            bqkvo[l, 3, :, 0] = Pm["bo"][l]
            for m in range(4):
                w1a[l, m] = Pm["W1"][l][:, m * 128:(m + 1) * 128]
                b1col[l, m, :, 0] = Pm["b1"][l][m * 128:(m + 1) * 128]
                w2a[l, m] = Pm["W2"][l][m * 128:(m + 1) * 128, :]
            b2acol[l, :, 0] = Pm["b2"][l]
        postW = np.stack([Pm["post_W"][:, m * 128:(m + 1) * 128]
                          for m in range(3)])
        postb = np.stack([Pm["post_b"][m * 128:(m + 1) * 128, None]
                          for m in range(3)])
        oh_atok = np.zeros((2, 128, N_TOKEN), np.float32)
        for a in range(NA):
            oh_atok[a // 128, a % 128, tloc[a]] = 1.0

        def b16(x):
            return np.ascontiguousarray(x).astype(bf)

        m = {
            "xinT": b16(xinT), "aW": b16(aW), "tsW": b16(tsW),
            "oh_uloc": b16(oh_uloc),
            "bemb": Pm["atom_b"][:, None].astype(np.float32),
            "posneg": b16(posneg), "pairsc": pairsc, "wa_bd": b16(wa_bd),
            "bacol": bacol.astype(np.float32), "wm2": b16(wm2),
            "trunkW": b16(trunkW),
            "oh_u": b16(np.stack([onehot_u[0:128], onehot_u[128:256]])),
            "b2col": b2col.astype(np.float32),
            "ident": b16(identity), "identf": identity,
            "kvidx": kvidx, "pinnb": b16(pinnb), "trunknbT": b16(trunknbT),
            "wanb": b16(Wa), "ba64": ba[:, None].astype(np.float32),
            "wm2nb": b16(W2p), "wbL": b16(wbL), "bbcol": bbcol,
            "lncols": lncols,
            "wq": b16(wqa), "wk": b16(wka), "wv": b16(wva), "wo": b16(woa),
            "w1": b16(w1a), "w2": b16(w2a),
            "bqkvo": bqkvo, "b1col": b1col, "b2acol": b2acol,
            "postW": b16(postW), "postb": np.ascontiguousarray(postb),
            "oh_atok": b16(oh_atok),
        }
        in_maps.append(m)
        hosts.append(None)

    import os
    trace = os.environ.get("KTRACE", "0") == "1"
    res = run_bass_kernel_spmd(nc, in_maps, core_ids=list(range(N_CORES)),
                               trace=trace)
    if trace:
        _CACHE["exec_ns"] = res.exec_time_ns
        _CACHE["scopes"] = res.per_core_scope_times
    R = res.results

    p_lm = np.empty((N_ATOM, N_ATOM, 32), np.float32)
    c_atom0 = np.empty((N_ATOM, 128), np.float32)
    q_atom = np.empty((N_ATOM, 128), np.float32)
    for r in range(N_CORES):
        A = slice(r * NA, (r + 1) * NA)
        p_lm[A] = R[r]["plm"].reshape(NA, 32, N_ATOM).transpose(0, 2, 1)
        c_atom0[A] = R[r]["catom0T"].reshape(128, NA).T
        q_atom[A] = R[r]["qatomT"].reshape(128, NA).T
    a_token = R[0]["atok"].reshape(N_TOKEN, 384)
    return a_token, q_atom, c_atom0, p_lm


def wrap_idx(vals):
    n = len(vals)
    out = np.zeros((16, n // 16), np.int16)
    out[np.arange(n) % 16, np.arange(n) // 16] = vals
    return np.tile(out, (8, 1))


# revision 16
# speedup vs baseline: 1.1580x; 1.1580x over previous
"""AtomAttentionEncoder distributed Bass kernel for 8 TRN2 NeuronCores.

Atoms sharded 256/core. Channel-major (c-major) residual stream; pair MLP with
fused one-hot trunk add; block-sparse attention with all-gathered k/v and
dma_gather neighbor fetch; scatter-mean via one-hot matmul + AllReduce.
Outputs p_lm in [atom, c, j] layout (host transposes), c_atom0/q_atom
transposed on host.
"""
import numpy as np
import ml_dtypes

import concourse.bass as bass
import concourse.bacc as bacc
import concourse.mybir as mybir
import concourse.tile as tile
from concourse.bass_utils import run_bass_kernel_spmd

N_CORES = 8
N_ATOM, N_TOKEN, BLOCK = 2048, 256, 24
H, L, DH = 4, 3, 32
NA = N_ATOM // N_CORES          # 256 atoms/core
NBF = 128 * BLOCK               # 3072 neighbor pairs per atom-tile

FP32 = mybir.dt.float32
BF16 = mybir.dt.bfloat16
I16 = mybir.dt.int16
AF = mybir.ActivationFunctionType
ALU = mybir.AluOpType
AX = mybir.AxisListType

bf = ml_dtypes.bfloat16


def build():
    nc = bacc.Bacc("TRN2", target_bir_lowering=False, debug=False,
                   num_devices=N_CORES)
    P = {}
    specs = [
        ("xinT", (2, 128, NA), BF16), ("aW", (2, 128, 128), BF16),
        ("tsW", (2, 128, 128), BF16), ("oh_uloc", (2, 128, NA), BF16),
        ("bemb", (128, 1), FP32),
        ("posneg", (128, N_ATOM), BF16), ("pairsc", (128, NA // 32), FP32),
        ("wa_bd", (16, 128, 128), BF16), ("bacol", (128, 1), FP32),
        ("wm2", (2, 128, 128), BF16),
        ("trunkW", (2, 128, NA * 32), BF16), ("oh_u", (2, 128, N_ATOM), BF16),
        ("b2col", (128, 1), FP32), ("ident", (128, 128), BF16),
        ("identf", (128, 128), FP32),
        ("kvidx", (2, 128, NBF // 16), I16),
        ("pinnb", (4, 2 * NBF), BF16), ("trunknbT", (32, 2 * NBF), BF16),
        ("wanb", (4, 64), BF16), ("ba64", (64, 1), FP32),
        ("wm2nb", (64, 32), BF16),
        ("wbL", (32, 12), BF16), ("bbcol", (12, 1), FP32),
        ("lncols", (L, 4, 128, 1), FP32),
        ("wq", (L, 128, 128), BF16), ("wk", (L, 128, 128), BF16),
        ("wv", (L, 128, 128), BF16), ("wo", (L, 128, 128), BF16),
        ("w1", (L, 4, 128, 128), BF16), ("w2", (L, 4, 128, 128), BF16),
        ("bqkvo", (L, 4, 128, 1), FP32), ("b1col", (L, 4, 128, 1), FP32),
        ("b2acol", (L, 128, 1), FP32),
        ("postW", (3, 128, 128), BF16), ("postb", (3, 128, 1), FP32),
        ("oh_atok", (2, 128, N_TOKEN), BF16),
    ]
    for name, shape, dt in specs:
        P[name] = nc.declare_dram_parameter(name, list(shape), dt, isOutput=False)

    plm = nc.declare_dram_parameter("plm", [NA, 32, N_ATOM], FP32, isOutput=True)
    catom0T = nc.declare_dram_parameter("catom0T", [128, NA], FP32, isOutput=True)
    qatomT = nc.declare_dram_parameter("qatomT", [128, NA], FP32, isOutput=True)
    atok = nc.declare_dram_parameter("atok", [N_TOKEN, 384], FP32, isOutput=True)

    with tile.TileContext(nc, num_cores=N_CORES) as tc:
        with (
            tc.tile_pool(name="cst", bufs=1) as cst,
            tc.tile_pool(name="sb", bufs=2) as sb,
            tc.tile_pool(name="ps", bufs=2, space="PSUM") as ps,
            tc.tile_pool(name="dram", bufs=1, space="DRAM") as dram,
        ):
            C = {}
            TRANSIENT = {"pinnb", "trunknbT"}
            SKIP = {"trunkW"}
            for name, shape, dt in specs:
                if name in SKIP:
                    continue
                if len(shape) == 2:
                    pool_ = sb if name in TRANSIENT else cst
                    if name in TRANSIENT:
                        t = pool_.tile(list(shape), dt, name=f"c_{name}",
                                       tag="nbw", bufs=3)
                    else:
                        t = pool_.tile(list(shape), dt, name=f"c_{name}",
                                       bufs=1)
                    nc.sync.dma_start(t[:], P[name][:])
                    C[name] = t
                elif len(shape) == 3:
                    C[name] = []
                    for i in range(shape[0]):
                        t = cst.tile(list(shape[1:]), dt, name=f"c_{name}{i}")
                        nc.sync.dma_start(t[:], P[name][i])
                        C[name].append(t)
                else:
                    C[name] = []
                    for i in range(shape[0]):
                        row = []
                        for j in range(shape[1]):
                            t = cst.tile(list(shape[2:]), dt,
                                         name=f"c_{name}{i}_{j}")
                            nc.sync.dma_start(t[:], P[name][i, j])
                            row.append(t)
                        C[name].append(row)

            # ============ embed: c_atom0 ============
            cat_ps = ps.tile([128, NA], FP32, name="cat_ps", tag="gen")
            nc.tensor.matmul(cat_ps[:], C["aW"][0][:], C["xinT"][0][:], start=True,
                             stop=False)
            nc.tensor.matmul(cat_ps[:], C["aW"][1][:], C["xinT"][1][:], start=False,
                             stop=False)
            nc.tensor.matmul(cat_ps[:], C["tsW"][0][:], C["oh_uloc"][0][:],
                             start=False, stop=False)
            nc.tensor.matmul(cat_ps[:], C["tsW"][1][:], C["oh_uloc"][1][:],
                             start=False, stop=True)
            x_t = cst.tile([128, NA], FP32, name="x_t")
            nc.scalar.activation(x_t[:], cat_ps[:], AF.Identity,
                                 bias=C["bemb"][:])
            nc.sync.dma_start(catom0T[:], x_t[:])

            # ============ p_nb MLP + bias (c-major, then transpose) ============
            h1nb = sb.tile([64, 2 * NBF], BF16, name="h1nb", tag="nbw", bufs=3)
            for ch in range(12):
                sl = slice(ch * 512, (ch + 1) * 512)
                hps = ps.tile([64, 512], FP32, name=f"hnb{ch}", tag="gen")
                nc.tensor.matmul(hps[:], C["wanb"][:], C["pinnb"][:, sl],
                                 start=True, stop=True)
                nc.scalar.activation(h1nb[:, sl], hps[:], AF.Relu,
                                     bias=C["ba64"][:])
            pnb_t = sb.tile([32, 2 * NBF], BF16, name="pnb_t", tag="nbw", bufs=3)
            for ch in range(12):
                sl = slice(ch * 512, (ch + 1) * 512)
                pps = ps.tile([32, 512], FP32, name=f"pnb{ch}", tag="gen")
                nc.tensor.matmul(pps[:], C["wm2nb"][:], h1nb[:, sl],
                                 start=True, stop=True)
                nc.vector.tensor_tensor(pnb_t[:, sl], pps[:],
                                        C["trunknbT"][:, sl], op=ALU.add)
            biasL = sb.tile([12, 2 * NBF], BF16, name="biasL", tag="nbw", bufs=3)
            for ch in range(12):
                sl = slice(ch * 512, (ch + 1) * 512)
                bps = ps.tile([12, 512], FP32, name=f"bL{ch}", tag="gen")
                nc.tensor.matmul(bps[:], C["wbL"][:], pnb_t[:, sl],
                                 start=True, stop=True)
                nc.scalar.activation(biasL[:, sl], bps[:], AF.Identity,
                                     bias=C["bbcol"][:])
            # transpose [12, 128]-chunks -> bias_am [tile][128, 24, 12]
            bias_am = []
            for t in range(2):
                bam = cst.tile([128, BLOCK, 12], FP32, name=f"bam{t}")
                bias_am.append(bam)
                for b in range(BLOCK):
                    tb = ps.tile([128, 12], BF16, name=f"tb{t}_{b}", tag="gen")
                    nc.tensor.transpose(
                        tb[:], biasL[:, (t * BLOCK + b) * 128:
                                     (t * BLOCK + b) * 128 + 128],
                        C["ident"][0:12, 0:12])
                    nc.vector.tensor_copy(bam[:, b, :], tb[:])

            # ============ helpers ============
            def ln(x_in, scol, bcol, out16, pref):
                sq = sb.tile([128, NA], FP32, name=f"{pref}_sq", tag="lnsq")
                nc.vector.tensor_tensor(sq[:], x_in[:], x_in[:], op=ALU.mult)
                st = ps.tile([1, 2 * NA], FP32, name=f"{pref}_st", tag="gen")
                nc.tensor.matmul(st[:, 0:NA], onescol[:], x_in[:], start=True,
                                 stop=True)
                nc.tensor.matmul(st[:, NA:], onescol[:], sq[:], start=True,
                                 stop=True)
                stats = sb.tile([1, 2 * NA], FP32, name=f"{pref}_sts",
                                tag="lnsts")
                nc.vector.tensor_scalar(stats[:], st[:], 1.0 / 128, None,
                                        op0=ALU.mult)
                m2 = sb.tile([1, NA], FP32, name=f"{pref}_m2", tag="lnm2")
                nc.vector.tensor_tensor(m2[:], stats[:, 0:NA], stats[:, 0:NA],
                                        op=ALU.mult)
                var = sb.tile([1, NA], FP32, name=f"{pref}_var", tag="lnvar")
                nc.vector.tensor_tensor(var[:], stats[:, NA:], m2[:],
                                        op=ALU.subtract)
                sd = sb.tile([1, NA], FP32, name=f"{pref}_sd", tag="lnsd")
                nc.scalar.activation(sd[:], var[:], AF.Sqrt, bias=epscol[:])
                rstd = sb.tile([1, NA], FP32, name=f"{pref}_rs", tag="lnrs")
                nc.vector.reciprocal(rstd[:], sd[:])
                nm = sb.tile([1, NA], FP32, name=f"{pref}_nm", tag="lnnm")
                nc.vector.tensor_scalar(nm[:], stats[:, 0:NA], -1.0, None,
                                        op0=ALU.mult)
                rep = ps.tile([128, 2 * NA], FP32, name=f"{pref}_rep",
                              tag="gen")
                nc.tensor.matmul(rep[:, 0:NA], onesrow[:], nm[:], start=True,
                                 stop=True)
                nc.tensor.matmul(rep[:, NA:], onesrow[:], rstd[:], start=True,
                                 stop=True)
                xc = sb.tile([128, NA], FP32, name=f"{pref}_xc", tag="lnxc")
                nc.vector.tensor_tensor(xc[:], rep[:, 0:NA], x_in[:],
                                        op=ALU.add)
                xn = sb.tile([128, NA], FP32, name=f"{pref}_xn", tag="lnxn")
                nc.vector.tensor_tensor(xn[:], xc[:], rep[:, NA:], op=ALU.mult)
                nc.scalar.activation(out16[:], xn[:], AF.Identity,
                                     bias=bcol, scale=scol)

            epscol = cst.tile([1, 1], FP32, name="epscol")
            nc.gpsimd.memset(epscol[:], 1e-5)
            onescol = cst.tile([128, 1], FP32, name="onescol")
            nc.gpsimd.memset(onescol[:], 1.0)
            onesrow = cst.tile([1, 128], FP32, name="onesrow")
            nc.gpsimd.memset(onesrow[:], 1.0)

            def transpose_cm_to_am(src16, pref, n=2):
                """[128, NA] bf16 c-major -> list of [128,128] bf16 atom-major."""
                outs = []
                for t in range(n):
                    tp = ps.tile([128, 128], BF16, name=f"{pref}_tp{t}",
                                 tag="gen")
                    nc.tensor.transpose(tp[:], src16[:, t * 128:(t + 1) * 128],
                                        C["ident"][:])
                    o = sb.tile([128, 128], BF16, name=f"{pref}_am{t}",
                                tag=f"{pref}am")
                    nc.vector.tensor_copy(o[:], tp[:])
                    outs.append(o)
                return outs

            # kv DRAM buffers per layer
            kvb = [dram.tile([NA, 256], BF16, name=f"kvb{l}") for l in range(L)]
            kvf = [dram.tile([N_ATOM, 256], BF16, name=f"kvf{l}")
                   for l in range(L)]


            # ============ 3 attention layers ============
            LST = {}

            def produce(l):
                h16 = sb.tile([128, NA], BF16, name=f"h16_{l}", tag="h16")
                ln(x_t, C["lncols"][l][0][:], C["lncols"][l][1][:], h16, f"ln1_{l}")
                # qkv
                qt = sb.tile([128, NA], BF16, name=f"qt{l}", tag="qt")
                kt = sb.tile([128, NA], BF16, name=f"kt{l}", tag="kt")
                vt = sb.tile([128, NA], BF16, name=f"vt{l}", tag="vt")
                for (w, bi, dst) in ((C["wq"][l], 0, qt), (C["wk"][l], 1, kt),
                                     (C["wv"][l], 2, vt)):
                    pp = ps.tile([128, NA], FP32, name=f"qkv{l}_{bi}",
                                 tag="gen")
                    nc.tensor.matmul(pp[:], w[:], h16[:], start=True, stop=True)
                    nc.scalar.activation(dst[:], pp[:], AF.Identity,
                                         bias=C["bqkvo"][l][bi][:])
                q_am = transpose_cm_to_am(qt, f"q{l}")
                k_am = transpose_cm_to_am(kt, f"k{l}")
                v_am = transpose_cm_to_am(vt, f"v{l}")
                # assemble kv atom-major & bounce out
                for t in range(2):
                    kvt_ = sb.tile([128, 256], BF16, name=f"kvt{l}_{t}",
                                   tag="kvtas")
                    nc.vector.tensor_copy(kvt_[:, 0:128], k_am[t][:])
                    nc.vector.tensor_copy(kvt_[:, 128:256], v_am[t][:])
                    nc.sync.dma_start(kvb[l][t * 128:(t + 1) * 128, :], kvt_[:])
                nc.gpsimd.collective_compute(
                    "AllGather", ALU.bypass,
                    replica_groups=[list(range(N_CORES))],
                    ins=[kvb[l].opt()], outs=[kvf[l].opt()])
                gkvs = []
                for t in range(2):
                    gkv = sb.tile([128, BLOCK, 256], BF16,
                                  name=f"gkv{l}_{t}", tag=f"g{t}", bufs=2)
                    nc.gpsimd.dma_gather(gkv[:], kvf[l][:], C["kvidx"][t][:],
                                         num_idxs=NBF, num_idxs_reg=NBF,
                                         elem_size=256, single_packet=False)
                    gkvs.append(gkv)
                LST[l] = (q_am, gkvs)

            def consume(l):
                q_am, gkvs = LST[l]
                o_am = []
                for t in range(2):
                    gkv = gkvs[t]
                    prod = sb.tile([128, H, BLOCK, DH], BF16,
                                   name=f"pr{l}_{t}", tag="prod", bufs=1)
                    nc.vector.tensor_tensor(
                        prod[:].rearrange("p h b d -> p b h d"),
                        gkv[:, :, 0:128].rearrange("p b (h d) -> p b h d", h=H),
                        q_am[t][:].rearrange("p (o h d) -> p o h d", o=1,
                                             h=H).to_broadcast(
                            [128, BLOCK, H, DH]),
                        op=ALU.mult)
                    s1 = sb.tile([128, H * BLOCK], FP32, name=f"s1{l}_{t}",
                                 tag="s1")
                    nc.vector.tensor_reduce(
                        s1[:], prod[:].rearrange("p h b d -> p (h b) d"),
                        axis=AX.X, op=ALU.add)
                    sc = sb.tile([128, H, BLOCK], FP32, name=f"sc{l}_{t}",
                                 tag="sc")
                    nc.vector.tensor_tensor(
                        sc[:],
                        s1[:].rearrange("p (h b) -> p h b", h=H),
                        bias_am[t][:].rearrange("p b lh -> p lh b")[
                            :, l * 4:(l + 1) * 4, :],
                        op=ALU.add)
                    ex = sb.tile([128, H, BLOCK], FP32, name=f"ex{l}_{t}",
                                 tag="ex")
                    nc.scalar.activation(ex[:], sc[:], AF.Exp)
                    sm = sb.tile([128, H], FP32, name=f"sm{l}_{t}", tag="sm")
                    nc.vector.tensor_reduce(sm[:], ex[:], axis=AX.X, op=ALU.add)
                    rec = sb.tile([128, H], FP32, name=f"rc{l}_{t}", tag="rc")
                    nc.vector.reciprocal(rec[:], sm[:])
                    at = sb.tile([128, H, BLOCK], FP32, name=f"at{l}_{t}",
                                 tag="at")
                    nc.vector.tensor_tensor(
                        at[:], ex[:],
                        rec[:].rearrange("p (h o) -> p h o", o=1).to_broadcast(
                            [128, H, BLOCK]),
                        op=ALU.mult)
                    pr2 = sb.tile([128, H, DH, BLOCK], BF16, name=f"p2{l}_{t}",
                                  tag="prod", bufs=1)
                    nc.vector.tensor_tensor(
                        pr2[:].rearrange("p h d b -> p b h d"),
                        gkv[:, :, 128:256].rearrange("p b (h d) -> p b h d",
                                                     h=H),
                        at[:].rearrange("p h (b o) -> p b h o",
                                        o=1).to_broadcast(
                            [128, BLOCK, H, DH]),
                        op=ALU.mult)
                    oam = sb.tile([128, H * DH], FP32, name=f"oa{l}_{t}",
                                  tag="oam")
                    nc.vector.tensor_reduce(
                        oam[:], pr2[:].rearrange("p h d b -> p (h d) b"),
                        axis=AX.X, op=ALU.add)
                    o_am.append(oam)
                # o -> c-major, Wo, residual
                ocm = sb.tile([128, NA], BF16, name=f"ocm{l}", tag="ocm")
                for t in range(2):
                    op_ = ps.tile([128, 128], FP32, name=f"otp{l}_{t}",
                                  tag="gen")
                    nc.tensor.transpose(op_[:], o_am[t][:], C["identf"][:])
                    nc.vector.tensor_copy(ocm[:, t * 128:(t + 1) * 128], op_[:])
                rps = ps.tile([128, NA], FP32, name=f"rps{l}", tag="gen")
                nc.tensor.matmul(rps[:], C["wo"][l][:], ocm[:], start=True,
                                 stop=True)
                xb = sb.tile([128, NA], FP32, name=f"xb{l}", tag="xb")
                nc.scalar.activation(xb[:], rps[:], AF.Identity,
                                     bias=C["bqkvo"][l][3][:])
                nc.vector.tensor_tensor(x_t[:], x_t[:], xb[:], op=ALU.add)
                # mlp
                h2 = sb.tile([128, NA], BF16, name=f"h2_{l}", tag="h16")
                ln(x_t, C["lncols"][l][2][:], C["lncols"][l][3][:], h2, f"ln2_{l}")
                mh = sb.tile([128, 4 * NA], BF16, name=f"mh{l}", tag="mh")
                for m in range(4):
                    mp = ps.tile([128, NA], FP32, name=f"mp{l}_{m}", tag="gen")
                    nc.tensor.matmul(mp[:], C["w1"][l][m][:], h2[:], start=True,
                                     stop=True)
                    nc.scalar.activation(mh[:, m * NA:(m + 1) * NA], mp[:],
                                         AF.Relu, bias=C["b1col"][l][m][:])
                m2ps = ps.tile([128, NA], FP32, name=f"m2ps{l}", tag="gen")
                for k in range(4):
                    nc.tensor.matmul(m2ps[:], C["w2"][l][k][:],
                                     mh[:, k * NA:(k + 1) * NA],
                                     start=(k == 0), stop=(k == 3))
                xb2 = sb.tile([128, NA], FP32, name=f"xb2{l}", tag="xb")
                nc.scalar.activation(xb2[:], m2ps[:], AF.Identity,
                                     bias=C["b2acol"][l][:])
                nc.vector.tensor_tensor(x_t[:], x_t[:], xb2[:], op=ALU.add)

            # ============ pair pipeline (the bulk) ============
            def emit_pair(grp):
                pi = sb.tile([128, N_ATOM], BF16, name=f"pi{grp}", tag="pi")
                nc.vector.tensor_scalar(pi[0:96, :], C["posneg"][0:96, :],
                                        C["pairsc"][0:96, grp:grp + 1], None,
                                        op0=ALU.add)
                nc.vector.tensor_scalar(pi[96:128, :], C["posneg"][96:128, :],
                                        C["pairsc"][96:128, grp:grp + 1], None,
                                        op0=ALU.is_equal)
                for blk in range(8):
                    a0 = grp * 32 + blk * 4  # first atom (local) of block
                    csl0 = slice(a0 * 32, a0 * 32 + 128)
                    tw = []
                    for kc in range(2):
                        twt = sb.tile([128, 128], BF16,
                                      name=f"tw{grp}_{blk}_{kc}",
                                      tag=f"tw{kc}")
                        nc.sync.dma_start(twt[:], P["trunkW"][kc][:, csl0])
                        tw.append(twt)
                    for jc in range(4):
                        jsl = slice(jc * 512, (jc + 1) * 512)
                        h1a = ps.tile([128, 512], FP32,
                                      name=f"h1a{grp}_{blk}_{jc}", tag="h1a",
                                      bufs=2)
                        h1b = ps.tile([128, 512], FP32,
                                      name=f"h1b{grp}_{blk}_{jc}", tag="h1b",
                                      bufs=2)
                        nc.tensor.matmul(h1a[:], C["wa_bd"][blk * 2][:],
                                         pi[:, jsl], start=True, stop=True)
                        nc.tensor.matmul(h1b[:], C["wa_bd"][blk * 2 + 1][:],
                                         pi[:, jsl], start=True, stop=True)
                        h1as = sb.tile([128, 512], BF16,
                                       name=f"h1as{grp}_{blk}_{jc}", tag="h1as")
                        nc.scalar.activation(h1as[:], h1a[:], AF.Relu,
                                             bias=C["bacol"][:])
                        h1bs = sb.tile([128, 512], BF16,
                                       name=f"h1bs{grp}_{blk}_{jc}", tag="h1bs")
                        nc.scalar.activation(h1bs[:], h1b[:], AF.Relu,
                                             bias=C["bacol"][:])
                        p2 = ps.tile([128, 512], FP32,
                                     name=f"p2{grp}_{blk}_{jc}", tag="p2",
                                     bufs=2)
                        nc.tensor.matmul(p2[:], C["wm2"][0][:], h1as[:],
                                         start=True, stop=False)
                        nc.tensor.matmul(p2[:], C["wm2"][1][:], h1bs[:],
                                         start=False, stop=False)
                        nc.tensor.matmul(p2[:], tw[0][:], C["oh_u"][0][:, jsl],
                                         start=False, stop=False)
                        nc.tensor.matmul(p2[:], tw[1][:], C["oh_u"][1][:, jsl],
                                         start=False, stop=True)
                        po = sb.tile([128, 512], FP32,
                                     name=f"po{grp}_{blk}_{jc}", tag="po")
                        nc.vector.tensor_scalar(po[:], p2[:], C["b2col"][:],
                                                None, op0=ALU.add)
                        nc.sync.dma_start(
                            plm[:].rearrange("a c j -> (a c) j")[
                                a0 * 32:a0 * 32 + 128, jsl], po[:])

            produce(0)
            emit_pair(0)
            emit_pair(1)
            consume(0)
            produce(1)
            emit_pair(2)
            emit_pair(3)
            consume(1)
            produce(2)
            emit_pair(4)
            emit_pair(5)
            consume(2)
            emit_pair(6)
            emit_pair(7)

            # ============ outputs: q_atom, a_token ============
            nc.sync.dma_start(qatomT[:], x_t[:])
            x16f = sb.tile([128, NA], BF16, name="x16f")
            nc.vector.tensor_copy(x16f[:], x_t[:])
            qp_am = []
            for t in range(2):
                qpt = sb.tile([128, 385], BF16, name=f"qpam{t}")
                nc.vector.tensor_copy(qpt[:, 384:385],
                                      onescol[:].bitcast(FP32) if False
                                      else onescol[:])
                qp_am.append(qpt)
            for m in range(3):
                pp = ps.tile([128, NA], FP32, name=f"qproj{m}", tag="gen")
                nc.tensor.matmul(pp[:], C["postW"][m][:], x16f[:], start=True,
                                 stop=True)
                qps = sb.tile([128, NA], BF16, name=f"qps{m}", tag="qps")
                nc.scalar.activation(qps[:], pp[:], AF.Identity,
                                     bias=C["postb"][m][:])
                for t in range(2):
                    tp = ps.tile([128, 128], BF16, name=f"qtp{m}_{t}",
                                 tag="gen")
                    nc.tensor.transpose(tp[:], qps[:, t * 128:(t + 1) * 128],
                                        C["ident"][:])
                    nc.vector.tensor_copy(
                        qp_am[t][:, m * 128:(m + 1) * 128], tp[:])
            arb = dram.tile([2, 128, 385], FP32, name="arb")
            aro = dram.tile([2, 128, 385], FP32, name="aro")
            for mt in range(2):
                sp = ps.tile([128, 385], FP32, name=f"sump{mt}", tag="gen")
                for kc in range(2):
                    nc.tensor.matmul(
                        sp[:], C["oh_atok"][kc][:, mt * 128:(mt + 1) * 128],
                        qp_am[kc][:], start=(kc == 0), stop=(kc == 1))
                ss = sb.tile([128, 385], FP32, name=f"sums{mt}", tag="sums")
                nc.vector.tensor_copy(ss[:], sp[:])
                nc.sync.dma_start(arb[mt], ss[:])
            nc.gpsimd.collective_compute(
                "AllReduce", ALU.add, replica_groups=[list(range(N_CORES))],
                ins=[arb.opt()], outs=[aro.opt()])
            for mt in range(2):
                sr = sb.tile([128, 385], FP32, name=f"sred{mt}", tag="sums")
                nc.sync.dma_start(sr[:], aro[mt])
                cnt = sb.tile([128, 1], FP32, name=f"cnt{mt}", tag="cnt")
                nc.vector.tensor_scalar(cnt[:], sr[:, 384:385], 1.0, None,
                                        op0=ALU.max)
                rcc = sb.tile([128, 1], FP32, name=f"rcc{mt}", tag="rcc")
                nc.vector.reciprocal(rcc[:], cnt[:])
                av = sb.tile([128, 384], FP32, name=f"av{mt}", tag="av")
                nc.vector.tensor_scalar(av[:], sr[:, 0:384], rcc[:], None,
                                        op0=ALU.mult)
                nc.sync.dma_start(atok[mt * 128:(mt + 1) * 128, :], av[:])



    nc.compile()
    return nc


_CACHE = {}


def kernel(ref_pos, ref_charge, ref_element, ref_atom_name_chars, atom_to_token,
           restype, trunk_sing, trunk_pair, block_index, params):
    Pm = {k: np.asarray(v, np.float32) for k, v in params.items()}
    pos = np.asarray(ref_pos, np.float32)
    a2t = np.asarray(atom_to_token, np.int64).astype(np.int32)
    bidx = np.asarray(block_index, np.int64).astype(np.int32)
    tsing = np.asarray(trunk_sing, np.float32)
    tpair = np.asarray(trunk_pair, np.float32)
    inv = np.float32(1.0 / np.sqrt(DH))

    x_in = np.concatenate(
        [pos, np.asarray(ref_charge, np.float32)[:, None],
         np.asarray(ref_element, np.float32),
         np.asarray(ref_atom_name_chars, np.float32)], axis=1)  # [2048, 148]

    if "nc" not in _CACHE:
        _CACHE["nc"] = build()
    nc = _CACHE["nc"]

    # fused stage-a weights: Wa = pair_W @ mlp1_W  [4, 64], ba = pair_b@mlp1+b1
    Wa = (Pm["pair_W"] @ Pm["mlp1_W"]).astype(np.float32)
    ba = (Pm["pair_b"] @ Pm["mlp1_W"] + Pm["mlp1_b"]).astype(np.float32)
    W2p = Pm["mlp2_W"]  # [64, 32]
    b2p = Pm["mlp2_b"]

    wa_bd = np.zeros((16, 128, 128), np.float32)
    for p in range(16):
        i0, i1 = 2 * p, 2 * p + 1
        for c in range(4):
            wa_bd[p, c * 32 + i0, 0:64] = Wa[c]
            wa_bd[p, c * 32 + i1, 64:128] = Wa[c]
    bacol = np.tile(ba, 2)[:, None]
    wm2 = np.zeros((2, 128, 128), np.float32)
    wm2[0, 0:64, 0:32] = W2p
    wm2[0, 64:128, 32:64] = W2p
    wm2[1, 0:64, 64:96] = W2p
    wm2[1, 64:128, 96:128] = W2p
    b2col = np.tile(b2p, 4)[:, None]

    onehot_u = np.zeros((256, N_ATOM), np.float32)
    onehot_u[a2t, np.arange(N_ATOM)] = 1.0

    identity = np.eye(128, dtype=np.float32)

    in_maps = []
    hosts = []
    for r in range(N_CORES):
        A = slice(r * NA, (r + 1) * NA)
        al = np.arange(r * NA, (r + 1) * NA)
        x_loc = x_in[A]  # [256, 148]
        xinT = np.zeros((2, 128, NA), np.float32)
        aW = np.zeros((2, 128, 128), np.float32)
        xt = x_loc.T  # [148, 256]
        xinT[0, :, :] = xt[0:128]
        xinT[1, 0:20, :] = xt[128:148]
        aW[0] = Pm["atom_W"][0:128]
        aW[1, 0:20] = Pm["atom_W"][128:148]
        tsW = np.stack([tsing[0:128], tsing[128:256]])  # [2,128,128]
        oh_uloc = np.zeros((2, 128, NA), np.float32)
        tloc = a2t[A]
        for a in range(NA):
            u = tloc[a]
            oh_uloc[u // 128, u % 128, a] = 1.0
        posneg = np.zeros((128, N_ATOM), np.float32)
        for c in range(3):
            posneg[c * 32:(c + 1) * 32, :] = -pos[:, c][None, :]
        posneg[96:128, :] = a2t[None, :].astype(np.float32)
        pairsc = np.zeros((128, NA // 32), np.float32)
        for g in range(NA // 32):
            ga = al[g * 32:(g + 1) * 32]
            for c in range(3):
                pairsc[c * 32:(c + 1) * 32, g] = pos[ga, c]
            pairsc[96:128, g] = a2t[ga]
        trunkW = np.zeros((2, 128, NA * 32), np.float32)
        for a in range(NA):
            t_a = a2t[al[a]]
            blk_ = tpair[t_a]  # [256, 32]
            trunkW[0, :, a * 32:(a + 1) * 32] = blk_[0:128]
            trunkW[1, :, a * 32:(a + 1) * 32] = blk_[128:256]
        # neighbor indices, flat order m = t*3072 + b*128 + p
        kvidx = np.zeros((2, 128, NBF // 16), np.int16)
        pinnb = np.zeros((4, 2 * NBF), np.float32)
        trunknbT = np.zeros((32, 2 * NBF), np.float32)
        for t in range(2):
            atoms = al[t * 128:(t + 1) * 128]
            flat = np.empty(NBF, np.int64)
            for b in range(BLOCK):
                flat[b * 128:(b + 1) * 128] = bidx[atoms, b]
            kvidx[t] = wrap_idx(flat)
            cols = slice(t * NBF, (t + 1) * NBF)
            arep = np.tile(atoms, BLOCK)
            barr = np.repeat(np.arange(BLOCK), 128)
            nb = bidx[arep, barr]
            pinnb[0:3, cols] = (pos[arep] - pos[nb]).T
            pinnb[3, cols] = (a2t[arep] == a2t[nb]).astype(np.float32)
            trunknbT[:, cols] = tpair[a2t[arep], a2t[nb]].T + b2p[:, None]
        wbL = np.zeros((32, 12), np.float32)
        bbcol = np.zeros((12, 1), np.float32)
        for l in range(L):
            wbL[:, l * 4:(l + 1) * 4] = Pm["Wb"][l]
            bbcol[l * 4:(l + 1) * 4, 0] = Pm["bb"][l]
        lncols = np.zeros((L, 4, 128, 1), np.float32)
        bqkvo = np.zeros((L, 4, 128, 1), np.float32)
        b1col = np.zeros((L, 4, 128, 1), np.float32)
        b2acol = np.zeros((L, 128, 1), np.float32)
        wqa = np.zeros((L, 128, 128), np.float32)
        wka = np.zeros((L, 128, 128), np.float32)
        wva = np.zeros((L, 128, 128), np.float32)
        woa = np.zeros((L, 128, 128), np.float32)
        w1a = np.zeros((L, 4, 128, 128), np.float32)
        w2a = np.zeros((L, 4, 128, 128), np.float32)
        for l in range(L):
            lncols[l, 0, :, 0] = Pm["ln1_s"][l]
            lncols[l, 1, :, 0] = Pm["ln1_b"][l]
            lncols[l, 2, :, 0] = Pm["ln2_s"][l]
            lncols[l, 3, :, 0] = Pm["ln2_b"][l]
            wqa[l] = Pm["Wq"][l] * inv
            wka[l] = Pm["Wk"][l]
            wva[l] = Pm["Wv"][l]
            woa[l] = Pm["Wo"][l]
            bqkvo[l, 0, :, 0] = Pm["bq"][l] * inv
            bqkvo[l, 1, :, 0] = Pm["bk"][l]
            bqkvo[l, 2, :, 0] = Pm["bv"][l]
            bqkvo[l, 3, :, 0] = Pm["bo"][l]
            for m in range(4):
                w1a[l, m] = Pm["W1"][l][:, m * 128:(m + 1) * 128]
                b1col[l, m, :, 0] = Pm["b1"][l][m * 128:(m + 1) * 128]
                w2a[l, m] = Pm["W2"][l][m * 128:(m + 1) * 128, :]
            b2acol[l, :, 0] = Pm["b2"][l]
        postW = np.stack([Pm["post_W"][:, m * 128:(m + 1) * 128]
                          for m in range(3)])
        postb = np.stack([Pm["post_b"][m * 128:(m + 1) * 128, None]
                          for m in range(3)])
        oh_atok = np.zeros((2, 128, N_TOKEN), np.float32)
        for a in range(NA):
            oh_atok[a // 128, a % 128, tloc[a]] = 1.0

        def b16(x):
            return np.ascontiguousarray(x).astype(bf)

        m = {
            "xinT": b16(xinT), "aW": b16(aW), "tsW": b16(tsW),
            "oh_uloc": b16(oh_uloc),
            "bemb": Pm["atom_b"][:, None].astype(np.float32),
            "posneg": b16(posneg), "pairsc": pairsc, "wa_bd": b16(wa_bd),
            "bacol": bacol.astype(np.float32), "wm2": b16(wm2),
            "trunkW": b16(trunkW),
            "oh_u": b16(np.stack([onehot_u[0:128], onehot_u[128:256]])),
            "b2col": b2col.astype(np.float32),
            "ident": b16(identity), "identf": identity,
            "kvidx": kvidx, "pinnb": b16(pinnb), "trunknbT": b16(trunknbT),
            "wanb": b16(Wa), "ba64": ba[:, None].astype(np.float32),
            "wm2nb": b16(W2p), "wbL": b16(wbL), "bbcol": bbcol,
            "lncols": lncols,
            "wq": b16(wqa), "wk": b16(wka), "wv": b16(wva), "wo": b16(woa),
            "w1": b16(w1a), "w2": b16(w2a),
            "bqkvo": bqkvo, "b1col": b1col, "b2acol": b2acol,
            "postW": b16(postW), "postb": np.ascontiguousarray(postb),
            "oh_atok": b16(oh_atok),
        }
        in_maps.append(m)
        hosts.append(None)

    import os
    trace = os.environ.get("KTRACE", "0") == "1"
    res = run_bass_kernel_spmd(nc, in_maps, core_ids=list(range(N_CORES)),
                               trace=trace)
    if trace:
        _CACHE["exec_ns"] = res.exec_time_ns
        _CACHE["scopes"] = res.per_core_scope_times
    R = res.results

    p_lm = np.empty((N_ATOM, N_ATOM, 32), np.float32)
    c_atom0 = np.empty((N_ATOM, 128), np.float32)
    q_atom = np.empty((N_ATOM, 128), np.float32)
    for r in range(N_CORES):
        A = slice(r * NA, (r + 1) * NA)
        p_lm[A] = R[r]["plm"].reshape(NA, 32, N_ATOM).transpose(0, 2, 1)
        c_atom0[A] = R[r]["catom0T"].reshape(128, NA).T
        q_atom[A] = R[r]["qatomT"].reshape(128, NA).T
    a_token = R[0]["atok"].reshape(N_TOKEN, 384)
    return a_token, q_atom, c_atom0, p_lm


def wrap_idx(vals):
    n = len(vals)
    out = np.zeros((16, n // 16), np.int16)
    out[np.arange(n) % 16, np.arange(n) // 16] = vals
    return np.tile(out, (8, 1))
